# revision 12
# baseline (speedup 1.0000x reference)
"""Trainium2 Bass kernel for nn_Net_16174846837292 (NNConv GNN message passing).

Strategy (graph-sharded, aggregation-folded, single fp16 a2 pass):
  pooled[g,o] = sum_{e: batch[dst[e]]=g} w_e * msg[e,o],  w_e = 1/max(cnt[dst_e],1)
  msg[e,o]    = sum_{k,i} e3[e,k]*h[src_e,i]*e4w[k,i*128+o] + sum_i h[src_e,i]*e4b[i*128+o]
  => pooled[g,o] = sum_k ZG_g[:,k]^T A2f[:,k*128+o] + HW_g^T Br
     ZG_g[i,k] = sum_{e in g} (w_e h[src_e,i]) e3[e,k],  HW_g[i] = sum_e w_e h[src_e,i]

Sharding: edges grouped by the graph of their destination node; 8 graphs per
core, so in-degree weights are per-edge host constants and NO collectives are
needed. Per-core edges pack into 8 slots of 192 (64-aligned segments).

Host precomputes w_e and pre-gathers x[src_e] per edge slot, so the device
kernel needs no histogram, no h DRAM round-trip, no indirect DMA, and no PE
transposes: the last layer of each MLP is computed edge-major by using the
previous layer's activations as the matmul stationary operand. All PE
operands are 16-bit (fp32 matmuls cost two array passes); PSUM accumulation
and bias adds stay fp32. Inputs arrive in 4 packed blobs + one 4 MB a2
stream on a second DMA queue. The e4 contraction streams a2 as the moving
operand against 16-col stationaries [zh_g | zl_g] (fp16 hi/lo split of ZG,
lo pre-scaled by 2^10 to stay fp16-normal), col-tiled 4-wide across the PE
array. Measured error vs the fp32 reference: ~4e-4 of output scale.
"""

import numpy as np
from contextlib import ExitStack

import concourse.bass as bass
import concourse.tile as tile
from concourse import bacc, mybir
from concourse.bass_utils import run_bass_kernel_spmd

N_CORES = 8
N, E, G, H = 4096, 8192, 64, 128
NODE_DIM, EDGE_DIM = 11, 5
G_PER_CORE = G // N_CORES          # 8 graph slots per core
CAP = 192                          # edge slots per graph (64-aligned segments)
EP = G_PER_CORE * CAP              # 1536 edge slots per core
NT = EP // 128                     # 12 edge tiles per core
NCH = EP // 512                    # 3 512-wide chunks for the feature-major MLPs
NG4 = NT // 4                      # 3 groups of 4 tiles for the edge-major stage
COLT = True                        # col-tile the final contraction 4-wide

f32 = mybir.dt.float32
f16 = mybir.dt.float16
AF = mybir.ActivationFunctionType
OP = mybir.AluOpType

# wblob column map (f16 weights packed into one [128, 1024] blob)
W_P2, W_E2, W_E30, W_E31, W_BR, W_P1, W_E1 = 0, 128, 384, 512, 640, 768, 896
# bias32 column map ([128, 16] f32)
B_P1, B_E1, B_E2, B_WME = 0, 1, 2, 4


def _slot_segments(s):
    """(tile, p0, p1) segments of graph slot s in the (p, t) edge grid."""
    segs, a, end = [], s * CAP, (s + 1) * CAP
    while a < end:
        t, p0 = divmod(a, 128)
        take = min(128 - p0, end - a)
        segs.append((t, p0, p0 + take))
        a += take
    return segs


def _emit(nc, tc, io):
    es = ExitStack()
    const = es.enter_context(tc.tile_pool(name="const", bufs=1))
    big = es.enter_context(tc.tile_pool(name="big", bufs=1))
    work = es.enter_context(tc.tile_pool(name="work", bufs=4))
    psA = es.enter_context(tc.tile_pool(name="psA", bufs=2, space="PSUM"))
    psB = es.enter_context(tc.tile_pool(name="psB", bufs=2, space="PSUM"))
    psZ = es.enter_context(tc.tile_pool(name="psZ", bufs=2, space="PSUM"))
    psO = es.enter_context(tc.tile_pool(name="psO", bufs=1, space="PSUM"))
    psR = es.enter_context(tc.tile_pool(name="psR", bufs=1, space="PSUM"))

    with es:
        # a2 stream on its own DMA queue (scalar HWDGE); everything else on sync
        a2 = const.tile([128, 128 * H], f16, tag="a2")
        nc.scalar.dma_start(a2[:], io["a2h"][:, :])

        eaT = const.tile([EDGE_DIM, EP], f16, tag="eaT")
        nc.sync.dma_start(eaT[:], io["edge16"][0:EDGE_DIM, :])
        xsT = const.tile([NODE_DIM, EP], f16, tag="xsT")
        nc.sync.dma_start(xsT[:], io["edge16"][EDGE_DIM:EDGE_DIM + NODE_DIM, :])
        wblob = const.tile([128, 1024], f16, tag="wblob")
        nc.sync.dma_start(wblob[:], io["wblob"][:, :])
        bias32 = const.tile([128, 16], f32, tag="bias32")
        nc.sync.dma_start(bias32[:], io["bias32"][:, :])
        rows32 = const.tile([1, 1024], f32, tag="rows32")
        nc.sync.dma_start(rows32[:], io["rows32"][:, :])

        # broadcast per-output-column biases to all partitions (512-wide)
        ones_r = const.tile([1, 128], f32, tag="ones_r")
        nc.vector.memset(ones_r[:], 1.0)
        pbc = psA.tile([128, 512], f32, tag="mlp")
        nc.tensor.matmul(pbc[:], ones_r[:], rows32[:, 0:512], start=True, stop=True)
        p2bb = const.tile([128, 512], f32, tag="p2bb")
        nc.scalar.copy(p2bb[:], pbc[:])
        pbc2 = psA.tile([128, 512], f32, tag="mlp")
        nc.tensor.matmul(pbc2[:], ones_r[:], rows32[:, 512:1024], start=True,
                         stop=True)
        e3bb = const.tile([128, 512], f32, tag="e3bb")
        nc.scalar.copy(e3bb[:], pbc2[:])

        # ---- feature-major MLP interiors (epilogues split ACT/DVE) ----------
        relu1 = big.tile([128, EP], f16, tag="relu1")
        e1o = big.tile([128, EP], f16, tag="e1o")
        e2o0 = big.tile([128, EP], f16, tag="e2o0")
        e2o1 = big.tile([128, EP], f16, tag="e2o1")
        for q in range(NCH):
            sl = slice(q * 512, (q + 1) * 512)
            ps = psA.tile([128, 512], f32, tag="mlp")
            nc.tensor.matmul(ps[:], wblob[0:EDGE_DIM, W_E1:W_E1 + 128],
                             eaT[:, sl], start=True, stop=True)
            nc.scalar.activation(e1o[:, sl], ps[:], AF.Relu,
                                 bias=bias32[:, B_E1:B_E1 + 1])
            ps2 = psA.tile([128, 512], f32, tag="mlp")
            nc.tensor.matmul(ps2[:], wblob[0:NODE_DIM, W_P1:W_P1 + 128],
                             xsT[:, sl], start=True, stop=True)
            nc.vector.tensor_scalar(relu1[:, sl], ps2[:],
                                    bias32[:, B_P1:B_P1 + 1], 0.0,
                                    op0=OP.add, op1=OP.max)
        for m, e2o in enumerate((e2o0, e2o1)):
            for q in range(NCH):
                sl = slice(q * 512, (q + 1) * 512)
                ps = psA.tile([128, 512], f32, tag="mlp")
                nc.tensor.matmul(ps[:], wblob[:, W_E2 + m * 128:W_E2 + (m + 1) * 128],
                                 e1o[:, sl], start=True, stop=True)
                if q % 2 == 0:
                    nc.scalar.activation(e2o[:, sl], ps[:], AF.Relu,
                                         bias=bias32[:, B_E2 + m:B_E2 + m + 1])
                else:
                    nc.vector.tensor_scalar(e2o[:, sl], ps[:],
                                            bias32[:, B_E2 + m:B_E2 + m + 1],
                                            0.0, op0=OP.add, op1=OP.max)

        # ---- edge-major last layers, grouped 4 tiles per 512-wide epilogue --
        # h_big[e, t, i] = w_e * (relu1[:,e].T @ p2w + p2b)
        # e3x_big[e, t, k] = relu(e2o[:,e].T @ e3w + e3b); col H = 1.0
        h_big = big.tile([128, NT, H], f16, tag="hbig")
        e3x = big.tile([128, NT, H + 1], f16, tag="e3x")
        nc.gpsimd.memset(e3x[:, :, H:H + 1], 1.0)
        for g4 in range(NG4):
            psh = psB.tile([128, 512], f32, tag="he4")
            pse = psA.tile([128, 512], f32, tag="mlp")
            for j in range(4):
                t = g4 * 4 + j
                sl = slice(t * 128, (t + 1) * 128)
                jj = slice(j * 128, (j + 1) * 128)
                nc.tensor.matmul(psh[:, jj], relu1[:, sl], wblob[:, W_P2:W_P2 + 128],
                                 start=True, stop=True)
                nc.tensor.matmul(pse[:, jj], e2o0[:, sl], wblob[:, W_E30:W_E30 + 128],
                                 start=True, stop=False)
                nc.tensor.matmul(pse[:, jj], e2o1[:, sl], wblob[:, W_E31:W_E31 + 128],
                                 start=False, stop=True)
            h4 = work.tile([128, 512], f32, tag="h4")
            nc.vector.tensor_tensor(h4[:], psh[:], p2bb[:], op=OP.add)
            for j in range(4):
                t = g4 * 4 + j
                nc.gpsimd.tensor_scalar_mul(h_big[:, t, :],
                                            h4[:, j * 128:(j + 1) * 128],
                                            bias32[:, B_WME + t:B_WME + t + 1])
            t4 = work.tile([128, 512], f32, tag="h4")
            nc.vector.tensor_tensor(t4[:], pse[:], e3bb[:], op=OP.add)
            nc.vector.tensor_scalar_max(e3x[:, 4 * g4:4 * g4 + 4, 0:H], t4[:], 0.0)

        # ---- per-graph ZG accumulation + fp16 hi/lo split ---------------------
        # zg2[:, 0:8, k] = zh, zg2[:, 8:16, k] = 1024*zl (host scales back)
        zg2 = big.tile([128, 2 * G_PER_CORE, H], f16, tag="zg2")
        hw_f = work.tile([128, G_PER_CORE], f16, tag="hwf")
        for s in range(G_PER_CORE):
            segs = _slot_segments(s)
            pz = psZ.tile([128, H + 1], f32, tag="zg")
            for n, (t, p0, p1) in enumerate(segs):
                nc.tensor.matmul(pz[:], h_big[p0:p1, t, :], e3x[p0:p1, t, :],
                                 start=(n == 0), stop=(n == len(segs) - 1))
            nc.vector.tensor_copy(zg2[:, s, :], pz[:, 0:H])
            zhf = work.tile([128, H], f32, tag="zhf")
            nc.scalar.activation(zhf[:], zg2[:, s, :], AF.Copy, scale=1024.0)
            nc.vector.scalar_tensor_tensor(zg2[:, G_PER_CORE + s, :],
                                           pz[:, 0:H], 1024.0, zhf[:],
                                           op0=OP.mult, op1=OP.subtract)
            nc.vector.tensor_copy(hw_f[:, s:s + 1], pz[:, H:H + 1])

        # ---- final a2 contraction: a2 streams as the moving operand ----------
        ot = work.tile([128, 128], f32, tag="ot")
        nc.gpsimd.memset(ot[:], 0.0)
        if COLT:
            po = psO.tile([128, 128], f32, tag="out")
            for k4 in range(H // 4):
                for j in range(4):
                    k = k4 * 4 + j
                    nc.tensor.matmul(po[32 * j:32 * j + 16, :], zg2[:, :, k],
                                     a2[:, k * 128:(k + 1) * 128],
                                     start=(k4 == 0), stop=(k4 == H // 4 - 1),
                                     tile_position=(0, 32 * j))
            for j in range(4):
                nc.scalar.copy(ot[32 * j:32 * j + 16, :], po[32 * j:32 * j + 16, :])
        else:
            po = psO.tile([2 * G_PER_CORE, 128], f32, tag="out")
            for k in range(H):
                nc.tensor.matmul(po[:], zg2[:, :, k],
                                 a2[:, k * 128:(k + 1) * 128],
                                 start=(k == 0), stop=(k == H - 1))
            nc.scalar.copy(ot[0:2 * G_PER_CORE, :], po[:])
        pr = psR.tile([G_PER_CORE, 128], f32, tag="br")
        nc.tensor.matmul(pr[:], hw_f[:], wblob[:, W_BR:W_BR + 128],
                         start=True, stop=True)
        ot2 = work.tile([G_PER_CORE, 128], f32, tag="ot2")
        nc.scalar.copy(ot2[:], pr[:])
        nc.sync.dma_start(io["pooled"][0:128, :], ot[:])
        nc.sync.dma_start(io["pooled"][128:128 + G_PER_CORE, :], ot2[:])


_CACHE = {}


def _build():
    if "nc" in _CACHE:
        return _CACHE["nc"]
    nc = bacc.Bacc("TRN2", target_bir_lowering=False, debug=False,
                   num_devices=N_CORES)
    io = {}

    def din(name, shape, dt=f32):
        io[name] = nc.dram_tensor(name, shape, dt, kind="ExternalInput").ap()

    din("edge16", [16, EP], f16)
    din("wblob", [128, 1024], f16)
    din("bias32", [128, 16])
    din("rows32", [1, 1024])
    din("a2h", [128, 128 * H], f16)
    io["pooled"] = nc.dram_tensor("pooled", [128 + G_PER_CORE, H], f32,
                                  kind="ExternalOutput").ap()

    with tile.TileContext(nc) as tc:
        _emit(nc, tc, io)
    nc.compile()
    _CACHE["nc"] = nc
    return nc


def _host_prep(inputs):
    x = np.asarray(inputs["x"], dtype=np.float32)
    ea = np.asarray(inputs["edge_attr"], dtype=np.float32)
    ei = np.asarray(inputs["edge_index"]).astype(np.int64)
    batch = np.asarray(inputs["batch"]).astype(np.int64)
    src, dst = ei[0], ei[1]
    gid = batch[dst]
    cnt = np.bincount(dst, minlength=N).astype(np.float32)
    w_all = 1.0 / np.maximum(cnt, 1.0)

    a2h = np.ascontiguousarray(
        np.asarray(inputs["e4_w"], np.float32)
        .reshape(128, 128, 128).transpose(1, 0, 2).reshape(128, 128 * H)
        .astype(np.float16))

    wblob = np.zeros((128, 1024), np.float16)
    wblob[:, W_P2:W_P2 + 128] = np.asarray(inputs["p2_w"], np.float16)
    wblob[:, W_E2:W_E2 + 256] = np.asarray(inputs["e2_w"], np.float16)
    wblob[:, W_E30:W_E30 + 128] = np.asarray(inputs["e3_w"], np.float16)[0:128]
    wblob[:, W_E31:W_E31 + 128] = np.asarray(inputs["e3_w"], np.float16)[128:256]
    wblob[:, W_BR:W_BR + 128] = (
        np.asarray(inputs["e4_b"], np.float32).reshape(128, 128).astype(np.float16))
    wblob[0:NODE_DIM, W_P1:W_P1 + 128] = np.asarray(inputs["p1_w"], np.float16)
    wblob[0:EDGE_DIM, W_E1:W_E1 + 128] = np.asarray(inputs["e1_w"], np.float16)

    rows32 = np.zeros((1, 1024), np.float32)
    rows32[0, 0:512] = np.tile(np.asarray(inputs["p2_b"], np.float32), 4)
    rows32[0, 512:1024] = np.tile(np.asarray(inputs["e3_b"], np.float32), 4)

    bias_c = np.zeros((128, 16), np.float32)
    bias_c[:, B_P1] = np.asarray(inputs["p1_b"], np.float32)
    bias_c[:, B_E1] = np.asarray(inputs["e1_b"], np.float32)
    bias_c[:, B_E2:B_E2 + 2] = np.asarray(
        inputs["e2_b"], np.float32).reshape(2, 128).T

    com = {"wblob": wblob, "rows32": rows32, "a2h": a2h}
    com = {k: np.ascontiguousarray(v) for k, v in com.items()}

    in_maps = []
    for c in range(N_CORES):
        ea_s = np.zeros((EP, EDGE_DIM), np.float32)
        xs_s = np.zeros((EP, NODE_DIM), np.float32)
        w_s = np.zeros(EP, np.float32)
        for s in range(G_PER_CORE):
            es = np.where(gid == c * G_PER_CORE + s)[0]
            assert len(es) <= CAP, f"graph {c * G_PER_CORE + s}: {len(es)} edges"
            pos = s * CAP + np.arange(len(es))
            ea_s[pos] = ea[es]
            xs_s[pos] = x[src[es]]
            w_s[pos] = w_all[dst[es]]

        edge16 = np.zeros((16, EP), np.float16)
        edge16[0:EDGE_DIM] = ea_s.T
        edge16[EDGE_DIM:EDGE_DIM + NODE_DIM] = xs_s.T
        b = bias_c.copy()
        b[:, B_WME:B_WME + NT] = w_s.reshape(NT, 128).T

        m = dict(com)
        m["edge16"] = np.ascontiguousarray(edge16)
        m["bias32"] = np.ascontiguousarray(b)
        in_maps.append(m)
    return in_maps


def _run(inputs, trace=False, tmpdir=None):
    nc = _build()
    in_maps = _host_prep(inputs)
    if trace:
        # No egress in this sandbox: neutralize the artifact upload the
        # trace path performs after NTFF capture, and register the NTFF
        # hook module if the image's antenv package lacks axon_hooks.
        from concourse import bass_utils as _bu
        _bu.upload_artifacts = lambda d: d
        try:
            from antenv import axon_hooks  # noqa: F401
        except ImportError:
            import importlib.util, sys as _sys
            spec = importlib.util.spec_from_file_location(
                "antenv.axon_hooks", "/opt/trn_rl_repo/antenv/axon_hooks.py")
            mod = importlib.util.module_from_spec(spec)
            spec.loader.exec_module(mod)
            _sys.modules["antenv.axon_hooks"] = mod
    res = run_bass_kernel_spmd(nc, in_maps, list(range(N_CORES)),
                               trace=trace, tmpdir=tmpdir)
    out = np.empty((G, H), np.float32)
    for c in range(N_CORES):
        p = res.results[c]["pooled"]
        acc = p[128:128 + G_PER_CORE].astype(np.float32).copy()
        ngrp = 4 if COLT else 1
        for j in range(ngrp):
            acc += p[32 * j:32 * j + G_PER_CORE]
            acc += p[32 * j + G_PER_CORE:32 * j + 2 * G_PER_CORE] * (1.0 / 1024.0)
        out[c * G_PER_CORE:(c + 1) * G_PER_CORE, :] = acc
    return out, res


def kernel(**inputs) -> np.ndarray:
    out, _ = _run(inputs)
    return out


# revision 13
# speedup vs baseline: 1.2555x; 1.2555x over previous
"""Trainium2 Bass kernel for nn_Net_16174846837292 (NNConv GNN message passing).

Strategy (graph-sharded, aggregation-folded, single fp16 a2 pass):
  pooled[g,o] = sum_{e: batch[dst[e]]=g} w_e * msg[e,o],  w_e = 1/max(cnt[dst_e],1)
  msg[e,o]    = sum_{k,i} e3[e,k]*h[src_e,i]*e4w[k,i*128+o] + sum_i h[src_e,i]*e4b[i*128+o]
  => pooled[g,o] = sum_k ZG_g[:,k]^T A2f[:,k*128+o] + HW_g^T Br
     ZG_g[i,k] = sum_{e in g} (w_e h[src_e,i]) e3[e,k],  HW_g[i] = sum_e w_e h[src_e,i]

Sharding: edges grouped by the graph of their destination node; 8 graphs per
core, so in-degree weights are per-edge host constants and NO collectives are
needed. Per-core edges pack into 8 slots of 192 (64-aligned segments).

Host precomputes w_e and pre-gathers x[src_e] per edge slot, so the device
kernel needs no histogram, no h DRAM round-trip, no indirect DMA, and no PE
transposes: the last layer of each MLP is computed edge-major by using the
previous layer's activations as the matmul stationary operand. All PE
operands are 16-bit (fp32 matmuls cost two array passes); PSUM accumulation
and bias adds stay fp32. Inputs arrive in 4 packed blobs + one 4 MB a2
stream on a second DMA queue. The e4 contraction streams a2 as the moving
operand against 16-col stationaries [zh_g | zl_g] (fp16 hi/lo split of ZG,
lo pre-scaled by 2^10 to stay fp16-normal), col-tiled 4-wide across the PE
array. Measured error vs the fp32 reference: ~4e-4 of output scale.
"""

import numpy as np
from contextlib import ExitStack

import concourse.bass as bass
import concourse.tile as tile
from concourse import bacc, mybir
from concourse.bass_utils import run_bass_kernel_spmd

N_CORES = 8
N, E, G, H = 4096, 8192, 64, 128
NODE_DIM, EDGE_DIM = 11, 5
G_PER_CORE = G // N_CORES          # 8 graph slots per core
CAP = 192                          # edge slots per graph (64-aligned segments)
EP = G_PER_CORE * CAP              # 1536 edge slots per core
NT = EP // 128                     # 12 edge tiles per core
NCH = EP // 512                    # 3 512-wide chunks for the feature-major MLPs
NG4 = NT // 4                      # 3 groups of 4 tiles for the edge-major stage
COLT = True                        # col-tile the final contraction 4-wide

f32 = mybir.dt.float32
f16 = mybir.dt.float16
AF = mybir.ActivationFunctionType
OP = mybir.AluOpType

# wblob column map (f16 weights packed into one [128, 1024] blob)
W_P2, W_E2, W_E30, W_E31, W_BR, W_P1, W_E1 = 0, 128, 384, 512, 640, 768, 896
# bias32 column map ([128, 16] f32)
B_P1, B_E1, B_E2, B_WME = 0, 1, 2, 4


def _slot_segments(s):
    """(tile, p0, p1) segments of graph slot s in the (p, t) edge grid."""
    segs, a, end = [], s * CAP, (s + 1) * CAP
    while a < end:
        t, p0 = divmod(a, 128)
        take = min(128 - p0, end - a)
        segs.append((t, p0, p0 + take))
        a += take
    return segs


def _emit(nc, tc, io):
    es = ExitStack()
    const = es.enter_context(tc.tile_pool(name="const", bufs=1))
    big = es.enter_context(tc.tile_pool(name="big", bufs=1))
    work = es.enter_context(tc.tile_pool(name="work", bufs=4))
    psA = es.enter_context(tc.tile_pool(name="psA", bufs=2, space="PSUM"))
    psB = es.enter_context(tc.tile_pool(name="psB", bufs=2, space="PSUM"))
    psZ = es.enter_context(tc.tile_pool(name="psZ", bufs=2, space="PSUM"))
    psO = es.enter_context(tc.tile_pool(name="psO", bufs=1, space="PSUM"))
    psR = es.enter_context(tc.tile_pool(name="psR", bufs=1, space="PSUM"))

    with es:
        # small loads first on sync; the 4 MB a2 split across the sync and
        # gpsimd queues AFTER them (a queued DMA blocks its engine's
        # instruction stream until the transfer completes)
        a2 = const.tile([128, 128 * H], f16, tag="a2")
        eaT = const.tile([EDGE_DIM, EP], f16, tag="eaT")
        nc.sync.dma_start(eaT[:], io["edge16"][0:EDGE_DIM, :])
        xsT = const.tile([NODE_DIM, EP], f16, tag="xsT")
        nc.sync.dma_start(xsT[:], io["edge16"][EDGE_DIM:EDGE_DIM + NODE_DIM, :])
        wblob = const.tile([128, 1024], f16, tag="wblob")
        nc.sync.dma_start(wblob[:], io["wblob"][:, :])
        bias32 = const.tile([128, 16], f32, tag="bias32")
        nc.sync.dma_start(bias32[:], io["bias32"][:, :])
        rows32 = const.tile([1, 1024], f32, tag="rows32")
        nc.sync.dma_start(rows32[:], io["rows32"][:, :])
        HA2 = 64 * H
        nc.gpsimd.dma_start(a2[:, 0:HA2], io["a2h"][:, 0:HA2])
        nc.sync.dma_start(a2[:, HA2:2 * HA2], io["a2h"][:, HA2:2 * HA2])

        # broadcast per-output-column biases to all partitions (512-wide)
        ones_r = const.tile([1, 128], f32, tag="ones_r")
        nc.vector.memset(ones_r[:], 1.0)
        pbc = psA.tile([128, 512], f32, tag="mlp")
        nc.tensor.matmul(pbc[:], ones_r[:], rows32[:, 0:512], start=True, stop=True)
        p2bb = const.tile([128, 512], f32, tag="p2bb")
        nc.scalar.copy(p2bb[:], pbc[:])
        pbc2 = psA.tile([128, 512], f32, tag="mlp")
        nc.tensor.matmul(pbc2[:], ones_r[:], rows32[:, 512:1024], start=True,
                         stop=True)
        e3bb = const.tile([128, 512], f32, tag="e3bb")
        nc.scalar.copy(e3bb[:], pbc2[:])

        # ---- feature-major MLP interiors (epilogues split ACT/DVE) ----------
        relu1 = big.tile([128, EP], f16, tag="relu1")
        e1o = big.tile([128, EP], f16, tag="e1o")
        e2o0 = big.tile([128, EP], f16, tag="e2o0")
        e2o1 = big.tile([128, EP], f16, tag="e2o1")
        for q in range(NCH):
            sl = slice(q * 512, (q + 1) * 512)
            ps = psA.tile([128, 512], f32, tag="mlp")
            nc.tensor.matmul(ps[:], wblob[0:EDGE_DIM, W_E1:W_E1 + 128],
                             eaT[:, sl], start=True, stop=True)
            nc.scalar.activation(e1o[:, sl], ps[:], AF.Relu,
                                 bias=bias32[:, B_E1:B_E1 + 1])
            ps2 = psA.tile([128, 512], f32, tag="mlp")
            nc.tensor.matmul(ps2[:], wblob[0:NODE_DIM, W_P1:W_P1 + 128],
                             xsT[:, sl], start=True, stop=True)
            nc.vector.tensor_scalar(relu1[:, sl], ps2[:],
                                    bias32[:, B_P1:B_P1 + 1], 0.0,
                                    op0=OP.add, op1=OP.max)
        for m, e2o in enumerate((e2o0, e2o1)):
            for q in range(NCH):
                sl = slice(q * 512, (q + 1) * 512)
                ps = psA.tile([128, 512], f32, tag="mlp")
                nc.tensor.matmul(ps[:], wblob[:, W_E2 + m * 128:W_E2 + (m + 1) * 128],
                                 e1o[:, sl], start=True, stop=True)
                if q % 2 == 0:
                    nc.scalar.activation(e2o[:, sl], ps[:], AF.Relu,
                                         bias=bias32[:, B_E2 + m:B_E2 + m + 1])
                else:
                    nc.vector.tensor_scalar(e2o[:, sl], ps[:],
                                            bias32[:, B_E2 + m:B_E2 + m + 1],
                                            0.0, op0=OP.add, op1=OP.max)

        # ---- edge-major last layers, grouped 4 tiles per 512-wide epilogue --
        # h_big[e, t, i] = w_e * (relu1[:,e].T @ p2w + p2b)
        # e3x_big[e, t, k] = relu(e2o[:,e].T @ e3w + e3b); col H = 1.0
        h_big = big.tile([128, NT, H], f16, tag="hbig")
        e3x = big.tile([128, NT, H + 1], f16, tag="e3x")
        nc.gpsimd.memset(e3x[:, :, H:H + 1], 1.0)
        for g4 in range(NG4):
            psh = psB.tile([128, 512], f32, tag="he4")
            pse = psA.tile([128, 512], f32, tag="mlp")
            for j in range(4):
                t = g4 * 4 + j
                sl = slice(t * 128, (t + 1) * 128)
                jj = slice(j * 128, (j + 1) * 128)
                nc.tensor.matmul(psh[:, jj], relu1[:, sl], wblob[:, W_P2:W_P2 + 128],
                                 start=True, stop=True)
                nc.tensor.matmul(pse[:, jj], e2o0[:, sl], wblob[:, W_E30:W_E30 + 128],
                                 start=True, stop=False)
                nc.tensor.matmul(pse[:, jj], e2o1[:, sl], wblob[:, W_E31:W_E31 + 128],
                                 start=False, stop=True)
            h4 = work.tile([128, 512], f32, tag="h4")
            nc.vector.tensor_tensor(h4[:], psh[:], p2bb[:], op=OP.add)
            for j in range(4):
                t = g4 * 4 + j
                if j % 2 == 0:
                    nc.scalar.activation(h_big[:, t, :],
                                         h4[:, j * 128:(j + 1) * 128], AF.Copy,
                                         scale=bias32[:, B_WME + t:B_WME + t + 1])
                else:
                    nc.vector.tensor_scalar_mul(h_big[:, t, :],
                                                h4[:, j * 128:(j + 1) * 128],
                                                bias32[:, B_WME + t:B_WME + t + 1])
            t4 = work.tile([128, 512], f32, tag="h4")
            nc.vector.tensor_tensor(t4[:], pse[:], e3bb[:], op=OP.add)
            nc.vector.tensor_scalar_max(e3x[:, 4 * g4:4 * g4 + 4, 0:H], t4[:], 0.0)

        # ---- per-graph ZG accumulation + fp16 hi/lo split ---------------------
        # zg2[:, 0:8, k] = zh, zg2[:, 8:16, k] = 1024*zl (host scales back)
        zg2 = big.tile([128, 2 * G_PER_CORE, H], f16, tag="zg2")
        hw_f = work.tile([128, G_PER_CORE], f16, tag="hwf")
        for s in range(G_PER_CORE):
            segs = _slot_segments(s)
            pz = psZ.tile([128, H + 1], f32, tag="zg")
            for n, (t, p0, p1) in enumerate(segs):
                nc.tensor.matmul(pz[:], h_big[p0:p1, t, :], e3x[p0:p1, t, :],
                                 start=(n == 0), stop=(n == len(segs) - 1))
            nc.scalar.copy(zg2[:, s, :], pz[:, 0:H])
            zhf = work.tile([128, H], f32, tag="zhf")
            nc.scalar.activation(zhf[:], zg2[:, s, :], AF.Copy, scale=1024.0)
            nc.vector.scalar_tensor_tensor(zg2[:, G_PER_CORE + s, :],
                                           pz[:, 0:H], 1024.0, zhf[:],
                                           op0=OP.mult, op1=OP.subtract)
            nc.vector.tensor_copy(hw_f[:, s:s + 1], pz[:, H:H + 1])

        # ---- final a2 contraction: a2 streams as the moving operand ----------
        ot = work.tile([128, 128], f32, tag="ot")
        nc.gpsimd.memset(ot[:], 0.0)
        if COLT:
            po = psO.tile([128, 128], f32, tag="out")
            for k4 in range(H // 4):
                for j in range(4):
                    k = k4 * 4 + j
                    nc.tensor.matmul(po[32 * j:32 * j + 16, :], zg2[:, :, k],
                                     a2[:, k * 128:(k + 1) * 128],
                                     start=(k4 == 0), stop=(k4 == H // 4 - 1),
                                     tile_position=(0, 32 * j))
            for j in range(4):
                nc.scalar.copy(ot[32 * j:32 * j + 16, :], po[32 * j:32 * j + 16, :])
        else:
            po = psO.tile([2 * G_PER_CORE, 128], f32, tag="out")
            for k in range(H):
                nc.tensor.matmul(po[:], zg2[:, :, k],
                                 a2[:, k * 128:(k + 1) * 128],
                                 start=(k == 0), stop=(k == H - 1))
            nc.scalar.copy(ot[0:2 * G_PER_CORE, :], po[:])
        pr = psR.tile([G_PER_CORE, 128], f32, tag="br")
        nc.tensor.matmul(pr[:], hw_f[:], wblob[:, W_BR:W_BR + 128],
                         start=True, stop=True)
        ot2 = work.tile([G_PER_CORE, 128], f32, tag="ot2")
        nc.scalar.copy(ot2[:], pr[:])
        nc.sync.dma_start(io["pooled"][0:128, :], ot[:])
        nc.sync.dma_start(io["pooled"][128:128 + G_PER_CORE, :], ot2[:])


_CACHE = {}


def _build():
    if "nc" in _CACHE:
        return _CACHE["nc"]
    nc = bacc.Bacc("TRN2", target_bir_lowering=False, debug=False,
                   num_devices=N_CORES)
    io = {}

    def din(name, shape, dt=f32):
        io[name] = nc.dram_tensor(name, shape, dt, kind="ExternalInput").ap()

    din("edge16", [16, EP], f16)
    din("wblob", [128, 1024], f16)
    din("bias32", [128, 16])
    din("rows32", [1, 1024])
    din("a2h", [128, 128 * H], f16)
    io["pooled"] = nc.dram_tensor("pooled", [128 + G_PER_CORE, H], f32,
                                  kind="ExternalOutput").ap()

    with tile.TileContext(nc) as tc:
        _emit(nc, tc, io)
    nc.compile()
    _CACHE["nc"] = nc
    return nc


def _host_prep(inputs):
    x = np.asarray(inputs["x"], dtype=np.float32)
    ea = np.asarray(inputs["edge_attr"], dtype=np.float32)
    ei = np.asarray(inputs["edge_index"]).astype(np.int64)
    batch = np.asarray(inputs["batch"]).astype(np.int64)
    src, dst = ei[0], ei[1]
    gid = batch[dst]
    cnt = np.bincount(dst, minlength=N).astype(np.float32)
    w_all = 1.0 / np.maximum(cnt, 1.0)

    a2h = np.ascontiguousarray(
        np.asarray(inputs["e4_w"], np.float32)
        .reshape(128, 128, 128).transpose(1, 0, 2).reshape(128, 128 * H)
        .astype(np.float16))

    wblob = np.zeros((128, 1024), np.float16)
    wblob[:, W_P2:W_P2 + 128] = np.asarray(inputs["p2_w"], np.float16)
    wblob[:, W_E2:W_E2 + 256] = np.asarray(inputs["e2_w"], np.float16)
    wblob[:, W_E30:W_E30 + 128] = np.asarray(inputs["e3_w"], np.float16)[0:128]
    wblob[:, W_E31:W_E31 + 128] = np.asarray(inputs["e3_w"], np.float16)[128:256]
    wblob[:, W_BR:W_BR + 128] = (
        np.asarray(inputs["e4_b"], np.float32).reshape(128, 128).astype(np.float16))
    wblob[0:NODE_DIM, W_P1:W_P1 + 128] = np.asarray(inputs["p1_w"], np.float16)
    wblob[0:EDGE_DIM, W_E1:W_E1 + 128] = np.asarray(inputs["e1_w"], np.float16)

    rows32 = np.zeros((1, 1024), np.float32)
    rows32[0, 0:512] = np.tile(np.asarray(inputs["p2_b"], np.float32), 4)
    rows32[0, 512:1024] = np.tile(np.asarray(inputs["e3_b"], np.float32), 4)

    bias_c = np.zeros((128, 16), np.float32)
    bias_c[:, B_P1] = np.asarray(inputs["p1_b"], np.float32)
    bias_c[:, B_E1] = np.asarray(inputs["e1_b"], np.float32)
    bias_c[:, B_E2:B_E2 + 2] = np.asarray(
        inputs["e2_b"], np.float32).reshape(2, 128).T

    com = {"wblob": wblob, "rows32": rows32, "a2h": a2h}
    com = {k: np.ascontiguousarray(v) for k, v in com.items()}

    in_maps = []
    for c in range(N_CORES):
        ea_s = np.zeros((EP, EDGE_DIM), np.float32)
        xs_s = np.zeros((EP, NODE_DIM), np.float32)
        w_s = np.zeros(EP, np.float32)
        for s in range(G_PER_CORE):
            es = np.where(gid == c * G_PER_CORE + s)[0]
            assert len(es) <= CAP, f"graph {c * G_PER_CORE + s}: {len(es)} edges"
            pos = s * CAP + np.arange(len(es))
            ea_s[pos] = ea[es]
            xs_s[pos] = x[src[es]]
            w_s[pos] = w_all[dst[es]]

        edge16 = np.zeros((16, EP), np.float16)
        edge16[0:EDGE_DIM] = ea_s.T
        edge16[EDGE_DIM:EDGE_DIM + NODE_DIM] = xs_s.T
        b = bias_c.copy()
        b[:, B_WME:B_WME + NT] = w_s.reshape(NT, 128).T

        m = dict(com)
        m["edge16"] = np.ascontiguousarray(edge16)
        m["bias32"] = np.ascontiguousarray(b)
        in_maps.append(m)
    return in_maps


def _run(inputs, trace=False, tmpdir=None):
    nc = _build()
    in_maps = _host_prep(inputs)
    if trace:
        # No egress in this sandbox: neutralize the artifact upload the
        # trace path performs after NTFF capture, and register the NTFF
        # hook module if the image's antenv package lacks axon_hooks.
        from concourse import bass_utils as _bu
        _bu.upload_artifacts = lambda d: d
        try:
            from antenv import axon_hooks  # noqa: F401
        except ImportError:
            import importlib.util, sys as _sys
            spec = importlib.util.spec_from_file_location(
                "antenv.axon_hooks", "/opt/trn_rl_repo/antenv/axon_hooks.py")
            mod = importlib.util.module_from_spec(spec)
            spec.loader.exec_module(mod)
            _sys.modules["antenv.axon_hooks"] = mod
    res = run_bass_kernel_spmd(nc, in_maps, list(range(N_CORES)),
                               trace=trace, tmpdir=tmpdir)
    out = np.empty((G, H), np.float32)
    for c in range(N_CORES):
        p = res.results[c]["pooled"]
        acc = p[128:128 + G_PER_CORE].astype(np.float32).copy()
        ngrp = 4 if COLT else 1
        for j in range(ngrp):
            acc += p[32 * j:32 * j + G_PER_CORE]
            acc += p[32 * j + G_PER_CORE:32 * j + 2 * G_PER_CORE] * (1.0 / 1024.0)
        out[c * G_PER_CORE:(c + 1) * G_PER_CORE, :] = acc
    return out, res


def kernel(**inputs) -> np.ndarray:
    out, _ = _run(inputs)
    return out
